# revision 31
# baseline (speedup 1.0000x reference)
"""Trainium2 Bass kernel for nn_CausalSelfAttention_35931696398729.

Sharding: 8 cores = (batch b in {0,1}) x (kv-head n in {0..3}).
Each core computes its 4 query heads' causal GQA attention for its batch
plus the partial c_proj (rows of Wo for its heads); the host sums the 4
partials per batch.  No device collectives.

All matmul operands are fp16 (1 cyc/row in the cost model, like bf16,
with 8x lower quantization error).  PSUM stays f32.

Key structure:
 - qT/kT (d on partitions, t free) so scores come out as ST (keys on
   partitions, queries free) and PV consumes exp(ST) directly.
 - V is projected directly in [t, d] layout (x block as the stationary
   operand) -- no PE transposes.
 - QK RMSNorm: squares are taken from the PRE-RoPE psum (rotation
   preserves column norms).  q-side factor rq(t)/sqrt(HD) is multiplied
   into q during phase 1 (Pool partition_broadcast + DVE mul); k-side
   factor rk(s) rides the Exp activation's per-partition scale;
   gamma_q*gamma_k is folded into the K RoPE tables on the host.
 - softmax runs without max-subtraction but with a constant -2 bias in
   the exponent (softmax-invariant) so exp stays in fp16 range.
 - rowsum: P_acc += p on DVE (scalar_tensor_tensor, 4x mode), then one
   [1,TC] matmul per (head, q-chunk); 1/rowsum applied to OT columns.
 - c_proj for q-chunk qc-1 is interleaved into attention of qc at key-
   block granularity; y copies run on the Pool engine; y is fp16.
"""

import os
import sys

sys.path.insert(0, "/opt/trn_rl_repo")

import numpy as np

import concourse.bacc as bacc
import concourse.mybir as mybir
import concourse.tile as tile
from concourse import bass_utils

B, T, D = 2, 2048, 2048
NH, NKV, HD = 16, 4, 128
G = NH // NKV  # query heads per core
EPS = 1e-6
THETA = 10000.0
N_CORES = 8
P = 128
TC = 512            # q-chunk for attention / c_proj column chunk
NTC = T // TC       # 4
TC1 = 256           # t-chunk for phase-1 projections
NTC1 = T // TC1     # 8
NKT = D // P        # 16 contraction chunks
NTB = T // P        # 16 t-blocks
EXP_BIAS = -2.0     # constant exponent shift (softmax invariant)

F32 = mybir.dt.float32
DT = mybir.dt.float16
NP_DT = np.float16


def build_program():
    nc = bacc.Bacc("TRN2", target_bir_lowering=False, debug=False,
                   enable_asserts=False, num_devices=N_CORES)

    xT = nc.dram_tensor("xT", (D, T), DT, kind="ExternalInput").ap()
    # weights host-prearranged to SBUF layout (contiguous >=4KB DMA rows)
    wq = nc.dram_tensor("wq", (P, G, NKT, HD), DT, kind="ExternalInput").ap()
    wk = nc.dram_tensor("wk", (P, NKT, HD), DT, kind="ExternalInput").ap()
    wv = nc.dram_tensor("wv", (P, NKT, HD), DT, kind="ExternalInput").ap()
    wo = nc.dram_tensor("wo", (G * HD, D), DT, kind="ExternalInput").ap()
    cosq = nc.dram_tensor("cosq", (P, T), DT, kind="ExternalInput").ap()
    sinq = nc.dram_tensor("sinq", (P, T), DT, kind="ExternalInput").ap()
    cosk = nc.dram_tensor("cosk", (P, T), DT, kind="ExternalInput").ap()
    sink = nc.dram_tensor("sink", (P, T), DT, kind="ExternalInput").ap()
    y = nc.dram_tensor("y", (T, D), DT, kind="ExternalOutput").ap()

    AF = mybir.ActivationFunctionType
    ALU = mybir.AluOpType

    with tile.TileContext(nc) as tc, \
         nc.allow_low_precision(reason="fp16 matmul/softmax pipeline"):
        with tc.tile_pool(name="persist", bufs=1) as persist, \
             tc.tile_pool(name="stri", bufs=4) as stri:
            cosq_sb = persist.tile([P, T], DT)
            sinq_sb = persist.tile([P, T], DT)
            cosk_sb = persist.tile([P, T], DT)
            sink_sb = persist.tile([P, T], DT)
            ones_col = persist.tile([P, 1], DT)
            nc.vector.memset(ones_col, 1.0)
            warm_src = persist.tile([P, P], DT)
            nc.vector.memset(warm_src, 0.0)
            eps_k = persist.tile([P, 1], F32)
            nc.vector.memset(eps_k, EPS)
            eps_q = persist.tile([1, 1], F32)
            nc.vector.memset(eps_q, HD * EPS)
            expb = persist.tile([P, 1], F32)
            nc.vector.memset(expb, EXP_BIAS)

            q_sb = [persist.tile([P, T], DT, tag=f"q_sb{h}", name=f"q_sb{h}")
                    for h in range(G)]
            kT_sb = persist.tile([P, T], DT)
            v_sb = persist.tile([P, NTB, P], DT)
            rk_tiles = persist.tile([P, NTB], F32)
            wo_sb = persist.tile([P, G, D], DT)
            otn_sb = [persist.tile([P, T], DT, tag=f"otn{h}", name=f"otn{h}")
                      for h in range(G)]

            # PE warm-up: keep PE busy through the cold-clock ramp window
            # while the first x chunks stream in.
            with tc.tile_pool(name="warm", bufs=1, space="PSUM") as ps_w:
                warm_ps = ps_w.tile([1, P], F32)
                for _ in range(20):
                    nc.tensor.matmul(warm_ps, ones_col, warm_src,
                                     start=True, stop=True)

            # ---------------- Phase 1: projections + RoPE + norms -----------
            with tc.tile_pool(name="weights", bufs=1) as wpool, \
                 tc.tile_pool(name="xts", bufs=4) as xpool, \
                 tc.tile_pool(name="p1tmp", bufs=4) as tmpool, \
                 tc.tile_pool(name="p1q", bufs=8) as qpool, \
                 tc.tile_pool(name="p1ps", bufs=3, space="PSUM") as ps_a, \
                 tc.tile_pool(name="p1psv", bufs=2, space="PSUM") as ps_v, \
                 tc.tile_pool(name="p1sc", bufs=1, space="PSUM") as ps_sc, \
                 tc.tile_pool(name="p1sq", bufs=2, space="PSUM") as ps_sq:
                wq_sb = wpool.tile([P, G, NKT, HD], DT)
                wk_sb = wpool.tile([P, NKT, HD], DT)
                wv_sb = wpool.tile([P, NKT, HD], DT)
                nc.sync.dma_start(out=wk_sb, in_=wk)

                xts_tiles = {}

                def get_xts(i):
                    if i in xts_tiles or i >= NTC1:
                        return
                    xts = xpool.tile([P, NKT, TC1], DT, tag="xts",
                                     name="xts")
                    sl_i = slice(i * TC1, (i + 1) * TC1)
                    xre = xT[:, sl_i].rearrange("(kt p) m -> p kt m", p=P)
                    for kg in range(4):
                        nc.sync.dma_start(
                            out=xts[:, 4 * kg:4 * (kg + 1), :],
                            in_=xre[:, 4 * kg:4 * (kg + 1), :])
                    xts_tiles[i] = xts

                # startup staging on the SP DMA queue: PE-critical first,
                # tables behind the second x chunk (only DVE ropes wait)
                get_xts(0)
                for h in range(G):
                    nc.sync.dma_start(out=wq_sb[:, h], in_=wq[:, h])
                nc.sync.dma_start(out=wv_sb, in_=wv)
                get_xts(1)
                nc.sync.dma_start(out=cosk_sb, in_=cosk)
                nc.sync.dma_start(out=sink_sb, in_=sink)
                get_xts(2)
                nc.sync.dma_start(out=cosq_sb, in_=cosq)
                nc.sync.dma_start(out=sinq_sb, in_=sinq)

                def swap_copy(psb, tag):
                    # halves-swapped copy (single-input ops may cross
                    # partition bases; two-input SB+SB ops may not)
                    psb_sw = tmpool.tile([P, TC1], DT, tag=tag, name=tag)
                    nc.vector.tensor_copy(out=psb_sw[0:64, :],
                                          in_=psb[64:128, :])
                    nc.vector.tensor_copy(out=psb_sw[64:128, :],
                                          in_=psb[0:64, :])
                    return psb_sw

                def rope(psb, psb_sw, cos_t, sin_t, dst):
                    # dst = psb * cos + swap(psb) * sin   (all fp16 SBUF,
                    # partition-aligned; sin table carries the sign fold)
                    tmp = tmpool.tile([P, TC1], DT, tag="ropetmp",
                                      name="ropetmp")
                    nc.vector.tensor_mul(out=tmp, in0=psb_sw, in1=sin_t)
                    tmp2 = tmpool.tile([P, TC1], DT, tag="ropetmp2",
                                       name="ropetmp2")
                    nc.vector.tensor_mul(out=tmp2, in0=psb, in1=cos_t)
                    nc.vector.tensor_add(out=dst, in0=tmp2, in1=tmp)

                for tc_i in range(NTC1):
                    sl = slice(tc_i * TC1, (tc_i + 1) * TC1)
                    get_xts(tc_i + 1)
                    if tc_i == 3:
                        nc.sync.dma_start(
                            out=wo_sb,
                            in_=wo.rearrange("(h p) m -> p h m", p=P))
                    xts = xts_tiles.pop(tc_i)

                    # ---- PE: projections (K, Q heads, V) -------------------
                    ps_k = ps_a.tile([P, TC1], F32, tag="proj", name="ps_k")
                    for kt in range(NKT):
                        nc.tensor.matmul(ps_k, wk_sb[:, kt, :],
                                         xts[:, kt, :],
                                         start=(kt == 0), stop=(kt == NKT - 1))
                    # Act: psum -> fp16 SBUF copy + square (pre-RoPE norm)
                    psb_k = tmpool.tile([P, TC1], DT, tag="psb", name="psb_k")
                    nc.scalar.copy(out=psb_k, in_=ps_k)
                    psw_k = swap_copy(psb_k, "psw")
                    sq_k = tmpool.tile([P, TC1], DT, tag="sq", name="sq_k")
                    nc.scalar.square(out=sq_k, in_=psb_k)
                    rope(psb_k, psw_k, cosk_sb[:, sl], sink_sb[:, sl],
                         kT_sb[:, sl])

                    q_ps = []
                    for h in range(G):
                        ps_q = ps_a.tile([P, TC1], F32, tag="proj",
                                         name="ps_q")
                        for kt in range(NKT):
                            nc.tensor.matmul(
                                ps_q, wq_sb[:, h, kt, :],
                                xts[:, kt, :],
                                start=(kt == 0), stop=(kt == NKT - 1))
                        psb_q = qpool.tile([P, TC1], DT, tag="psbq",
                                           name="psb_q")
                        nc.scalar.copy(out=psb_q, in_=ps_q)
                        psw_q = swap_copy(psb_q, "pswq")
                        sq_q = qpool.tile([P, TC1], DT, tag="sqq",
                                          name="sq_q")
                        nc.scalar.square(out=sq_q, in_=psb_q)
                        qr = qpool.tile([P, TC1], DT, tag="ropeq", name="qr")
                        rope(psb_q, psw_q, cosq_sb[:, sl], sinq_sb[:, sl], qr)
                        q_ps.append((sq_q, qr))

                    # V directly in [t, d] layout: x block stationary
                    for i in range(TC1 // P):
                        ps_vt = ps_v.tile([P, P], F32, tag="vt", name="ps_vt")
                        for kt in range(NKT):
                            nc.tensor.matmul(
                                ps_vt, xts[:, kt, i * P:(i + 1) * P],
                                wv_sb[:, kt, :],
                                start=(kt == 0), stop=(kt == NKT - 1))
                        nc.scalar.copy(
                            out=v_sb[:, tc_i * (TC1 // P) + i, :], in_=ps_vt)

                    # ---- norm reductions (PE, end of chunk) ----------------
                    # K: per key-block column sums of sq_k (sq stationary)
                    kb0 = tc_i * (TC1 // P)
                    ssqc = ps_sc.tile([P, TC1 // P], F32, tag="ssqc",
                                      name="ssqc")
                    for i in range(TC1 // P):
                        nc.tensor.matmul(ssqc[:, i:i + 1],
                                         sq_k[:, i * P:(i + 1) * P],
                                         ones_col, start=True, stop=True)
                    nc.scalar.activation(
                        out=rk_tiles[:, kb0:kb0 + TC1 // P], in_=ssqc,
                        func=AF.Sqrt, bias=eps_k[:], scale=float(1.0 / HD))
                    nc.vector.reciprocal(
                        out=rk_tiles[:, kb0:kb0 + TC1 // P],
                        in_=rk_tiles[:, kb0:kb0 + TC1 // P])
                    # Q: rq = 1/sqrt(ssq + HD*eps) applied to q columns
                    for h in range(G):
                        sq_q, qr = q_ps[h]
                        ssq = ps_sq.tile([1, TC1], F32, tag="ssq",
                                         name="ssq")
                        nc.tensor.matmul(ssq, ones_col, sq_q,
                                         start=True, stop=True)
                        sq_s = stri.tile([1, TC1], F32, tag="sqs",
                                         name="sq_s")
                        nc.scalar.activation(out=sq_s, in_=ssq, func=AF.Sqrt,
                                             bias=eps_q[:], scale=1.0)
                        rq = stri.tile([1, TC1], DT, tag="rq", name="rq")
                        nc.vector.reciprocal(out=rq, in_=sq_s)
                        rqB = tmpool.tile([P, TC1], DT, tag="rqB", name="rqB")
                        nc.gpsimd.partition_broadcast(rqB, rq)
                        nc.vector.tensor_mul(out=q_sb[h][:, sl], in0=qr,
                                             in1=rqB)

            # ---------------- Phase 2: attention + c_proj -------------------
            with tc.tile_pool(name="attn", bufs=4) as apool, \
                 tc.tile_pool(name="pb", bufs=6) as ppool, \
                 tc.tile_pool(name="pacc", bufs=4) as accpool, \
                 tc.tile_pool(name="ysb", bufs=4) as ypool, \
                 tc.tile_pool(name="p2st", bufs=2, space="PSUM") as ps_st, \
                 tc.tile_pool(name="p2ot", bufs=2, space="PSUM") as ps_ot, \
                 tc.tile_pool(name="p3ya", bufs=1, space="PSUM") as ps_ya, \
                 tc.tile_pool(name="p3yb", bufs=1, space="PSUM") as ps_yb:

                def cproj_steps(qc):
                    # 8 emission closures for q-chunk qc's 4 t-blocks
                    steps = []
                    for tb in range(4 * qc, 4 * qc + 4):
                        for jg in (0, 2):
                            def step(tb=tb, jg=jg):
                                ya = ps_ya.tile([P, TC], F32, tag="ya",
                                                name="ya")
                                yb = ps_yb.tile([P, TC], F32, tag="yb",
                                                name="yb")
                                for h in range(G):
                                    lhs = otn_sb[h][:, tb * P:(tb + 1) * P]
                                    nc.tensor.matmul(
                                        ya, lhs,
                                        wo_sb[:, h, jg * TC:(jg + 1) * TC],
                                        start=(h == 0), stop=(h == G - 1))
                                    nc.tensor.matmul(
                                        yb, lhs,
                                        wo_sb[:, h,
                                              (jg + 1) * TC:(jg + 2) * TC],
                                        start=(h == 0), stop=(h == G - 1))
                                for j, yp in ((jg, ya), (jg + 1, yb)):
                                    y_sb = ypool.tile([P, TC], DT, tag="y_sb",
                                                      name="y_sb")
                                    if j % 2 == 0:
                                        nc.scalar.copy(out=y_sb, in_=yp)
                                    else:
                                        nc.vector.tensor_copy(out=y_sb,
                                                              in_=yp)
                                    nc.sync.dma_start(
                                        out=y[tb * P:(tb + 1) * P,
                                              j * TC:(j + 1) * TC],
                                        in_=y_sb)
                            steps.append(step)
                    return steps

                for qc in range(NTC):
                    qsl = slice(qc * TC, (qc + 1) * TC)
                    nkb = 4 * (qc + 1)
                    steps = cproj_steps(qc - 1) if qc > 0 else []
                    # interleave points: 2 mid-loop per pair + pair ends
                    mids = {max(1, nkb // 3), max(2, (2 * nkb) // 3)}
                    for pair in ((0, 1), (2, 3)):
                        ot_ps = {}
                        for h in pair:
                            ot_ps[h] = ps_ot.tile([P, TC], F32, tag="ot",
                                                  name="ot_ps")
                        acc = accpool.tile([P, 2, TC], DT, tag="acc",
                                           name="acc")
                        for kb in range(nkb):
                            r = kb - 4 * qc  # >=0 on diagonal blocks
                            c0 = max(r, 0) * P  # first valid q column
                            # both heads' scores into one psum pair-tile
                            st = ps_st.tile([P, 2, TC], F32, tag="st",
                                            name="st_ps")
                            for i, h in enumerate(pair):
                                nc.tensor.matmul(
                                    st[:, i, c0:],
                                    kT_sb[:, kb * P:(kb + 1) * P],
                                    q_sb[h][:, qc * TC + c0:(qc + 1) * TC],
                                    start=True, stop=True)
                            # one exp over both heads (shared rk scale)
                            p_ = ppool.tile([P, 2, TC], DT, tag="p",
                                            name="p_sb")
                            nc.scalar.activation(
                                out=p_[:, :, c0:], in_=st[:, :, c0:],
                                func=AF.Exp, bias=expb[:],
                                scale=rk_tiles[:, kb:kb + 1])
                            if r >= 0:
                                for i in range(2):
                                    # causal mask on the diagonal strip
                                    nc.gpsimd.affine_select(
                                        out=p_[:, i, c0:c0 + P],
                                        in_=p_[:, i, c0:c0 + P],
                                        pattern=[[1, P]],
                                        compare_op=ALU.is_ge,
                                        fill=0.0,
                                        base=0,
                                        channel_multiplier=-1)
                            if kb == 0:
                                nc.vector.tensor_copy(out=acc, in_=p_)
                            else:
                                nc.vector.tensor_add(
                                    out=acc[:, :, c0:],
                                    in0=acc[:, :, c0:],
                                    in1=p_[:, :, c0:])
                            for i, h in enumerate(pair):
                                nc.tensor.matmul(
                                    ot_ps[h][:, c0:], v_sb[:, kb, :],
                                    p_[:, i, c0:], start=(kb == 0),
                                    stop=(kb == nkb - 1))
                            if steps and (kb in mids):
                                steps.pop(0)()
                        # pair rowsum -> 1/rowsum -> normalize (rs tile
                        # borrows a st-pool buffer; scores are done)
                        rs = ps_st.tile([P, 2, TC], F32, tag="st",
                                        name="rs_ps")
                        for i in range(2):
                            nc.tensor.matmul(rs[0:1, i, :], ones_col,
                                             acc[:, i, :],
                                             start=True, stop=True)
                        recip = stri.tile([1, 2, TC], DT, tag="recip",
                                          name="recip")
                        nc.vector.reciprocal(out=recip, in_=rs[0:1, :, :])
                        recipB = apool.tile([P, 2, TC], DT, tag="recipB",
                                            name="recipB")
                        nc.gpsimd.partition_broadcast(recipB, recip)
                        if steps:
                            steps.pop(0)()
                        for i, h in enumerate(pair):
                            nc.vector.tensor_mul(out=otn_sb[h][:, qsl],
                                                 in0=ot_ps[h],
                                                 in1=recipB[:, i, :])
                        if steps:
                            steps.pop(0)()
                    while steps:
                        steps.pop(0)()
                # final chunk's c_proj
                for step in cproj_steps(NTC - 1):
                    step()

    nc.compile()
    return nc


_NC_CACHE = None


def _get_program():
    global _NC_CACHE
    if _NC_CACHE is None:
        _NC_CACHE = build_program()
    return _NC_CACHE


def _make_tables(pos, gamma2):
    half = HD // 2
    inv_freq = 1.0 / (THETA ** (np.arange(half, dtype=np.float64) / half))
    ang = (pos + np.arange(T, dtype=np.float64))[None, :] * inv_freq[:, None]
    cos = np.cos(ang)
    sin = np.sin(ang)
    cosq = np.concatenate([cos, cos], axis=0)
    sinq = np.concatenate([-sin, sin], axis=0)
    g2 = gamma2.astype(np.float64).reshape(P, 1)
    return (np.ascontiguousarray(cosq.astype(NP_DT)),
            np.ascontiguousarray(sinq.astype(NP_DT)),
            np.ascontiguousarray((cosq * g2).astype(NP_DT)),
            np.ascontiguousarray((sinq * g2).astype(NP_DT)))


def kernel(x, Wq, Wk, Wv, Wo, q_gamma, k_gamma, pos):
    x = np.asarray(x, dtype=np.float32)
    Wq = np.asarray(Wq, dtype=np.float32)
    Wk = np.asarray(Wk, dtype=np.float32)
    Wv = np.asarray(Wv, dtype=np.float32)
    Wo = np.asarray(Wo, dtype=np.float32)
    q_gamma = np.asarray(q_gamma, dtype=np.float32)
    k_gamma = np.asarray(k_gamma, dtype=np.float32)
    pos = int(np.asarray(pos))

    gamma2 = q_gamma * k_gamma
    cosq, sinq, cosk, sink = _make_tables(pos, gamma2)

    def st(a):
        return np.ascontiguousarray(a.astype(NP_DT))

    def wqr(a):
        # (D, G*HD) -> (P, G, NKT, HD): [p, h, kt, :] = a[kt*P + p, h*HD:]
        return st(a.reshape(NKT, P, G, HD).transpose(1, 2, 0, 3))

    def wkvr(a):
        # (D, HD) -> (P, NKT, HD)
        return st(a.reshape(NKT, P, HD).transpose(1, 0, 2))

    xTs = [st(x[b].T) for b in range(B)]
    in_maps = []
    for c in range(N_CORES):
        b, n = divmod(c, NKV)
        in_maps.append({
            "xT": xTs[b],
            "wq": wqr(Wq[:, n * G * HD:(n + 1) * G * HD]),
            "wk": wkvr(Wk[:, n * HD:(n + 1) * HD]),
            "wv": wkvr(Wv[:, n * HD:(n + 1) * HD]),
            "wo": st(Wo[n * G * HD:(n + 1) * G * HD, :]),
            "cosq": cosq,
            "sinq": sinq,
            "cosk": cosk,
            "sink": sink,
        })

    nc = _get_program()
    res = bass_utils.run_bass_kernel_spmd(nc, in_maps,
                                          core_ids=list(range(N_CORES)))
    out = np.zeros((B, T, D), dtype=np.float32)
    for c in range(N_CORES):
        b = c // NKV
        out[b] += res.results[c]["y"].astype(np.float32)
    return out


if __name__ == "__main__":
    build_program()
    print("program built OK")


# revision 32
# speedup vs baseline: 1.0156x; 1.0156x over previous
"""Trainium2 Bass kernel for nn_CausalSelfAttention_35931696398729.

Sharding: 8 cores = (batch b in {0,1}) x (kv-head n in {0..3}).
Each core computes its 4 query heads' causal GQA attention for its batch
plus the partial c_proj (rows of Wo for its heads); the host sums the 4
partials per batch.  No device collectives.

All matmul operands are fp16 (1 cyc/row in the cost model, like bf16,
with 8x lower quantization error).  PSUM stays f32.

Key structure:
 - qT/kT (d on partitions, t free) so scores come out as ST (keys on
   partitions, queries free) and PV consumes exp(ST) directly.
 - V is projected directly in [t, d] layout (x block as the stationary
   operand) -- no PE transposes.
 - QK RMSNorm: squares are taken from the PRE-RoPE psum (rotation
   preserves column norms).  q-side factor rq(t)/sqrt(HD) is multiplied
   into q during phase 1 (Pool partition_broadcast + DVE mul); k-side
   factor rk(s) rides the Exp activation's per-partition scale;
   gamma_q*gamma_k is folded into the K RoPE tables on the host.
 - softmax runs without max-subtraction but with a constant -2 bias in
   the exponent (softmax-invariant) so exp stays in fp16 range.
 - rowsum: P_acc += p on DVE (scalar_tensor_tensor, 4x mode), then one
   [1,TC] matmul per (head, q-chunk); 1/rowsum applied to OT columns.
 - c_proj for q-chunk qc-1 is interleaved into attention of qc at key-
   block granularity; y copies run on the Pool engine; y is fp16.
"""

import os
import sys

sys.path.insert(0, "/opt/trn_rl_repo")

import numpy as np

import concourse.bacc as bacc
import concourse.mybir as mybir
import concourse.tile as tile
from concourse import bass_utils

B, T, D = 2, 2048, 2048
NH, NKV, HD = 16, 4, 128
G = NH // NKV  # query heads per core
EPS = 1e-6
THETA = 10000.0
N_CORES = 8
P = 128
TC = 512            # q-chunk for attention / c_proj column chunk
NTC = T // TC       # 4
TC1 = 256           # t-chunk for phase-1 projections
NTC1 = T // TC1     # 8
NKT = D // P        # 16 contraction chunks
NTB = T // P        # 16 t-blocks
EXP_BIAS = -2.0     # constant exponent shift (softmax invariant)

F32 = mybir.dt.float32
DT = mybir.dt.float16
NP_DT = np.float16


def build_program():
    nc = bacc.Bacc("TRN2", target_bir_lowering=False, debug=False,
                   enable_asserts=False, num_devices=N_CORES)

    xT = nc.dram_tensor("xT", (D, T), DT, kind="ExternalInput").ap()
    # weights host-prearranged to SBUF layout (contiguous >=4KB DMA rows)
    wq = nc.dram_tensor("wq", (P, G, NKT, HD), DT, kind="ExternalInput").ap()
    wk = nc.dram_tensor("wk", (P, NKT, HD), DT, kind="ExternalInput").ap()
    wv = nc.dram_tensor("wv", (P, NKT, HD), DT, kind="ExternalInput").ap()
    wo = nc.dram_tensor("wo", (G * HD, D), DT, kind="ExternalInput").ap()
    cosq = nc.dram_tensor("cosq", (P, T), DT, kind="ExternalInput").ap()
    sinq = nc.dram_tensor("sinq", (P, T), DT, kind="ExternalInput").ap()
    cosk = nc.dram_tensor("cosk", (P, T), DT, kind="ExternalInput").ap()
    sink = nc.dram_tensor("sink", (P, T), DT, kind="ExternalInput").ap()
    y = nc.dram_tensor("y", (T, D), DT, kind="ExternalOutput").ap()

    AF = mybir.ActivationFunctionType
    ALU = mybir.AluOpType

    with tile.TileContext(nc) as tc, \
         nc.allow_low_precision(reason="fp16 matmul/softmax pipeline"):
        with tc.tile_pool(name="persist", bufs=1) as persist, \
             tc.tile_pool(name="stri", bufs=4) as stri:
            cosq_sb = persist.tile([P, T], DT)
            sinq_sb = persist.tile([P, T], DT)
            cosk_sb = persist.tile([P, T], DT)
            sink_sb = persist.tile([P, T], DT)
            ones_col = persist.tile([P, 1], DT)
            nc.vector.memset(ones_col, 1.0)
            warm_src = persist.tile([P, P], DT)
            nc.vector.memset(warm_src, 0.0)
            eps_k = persist.tile([P, 1], F32)
            nc.vector.memset(eps_k, EPS)
            eps_q = persist.tile([1, 1], F32)
            nc.vector.memset(eps_q, HD * EPS)
            expb = persist.tile([P, 1], F32)
            nc.vector.memset(expb, EXP_BIAS)

            q_sb = [persist.tile([P, T], DT, tag=f"q_sb{h}", name=f"q_sb{h}")
                    for h in range(G)]
            kT_sb = persist.tile([P, T], DT)
            v_sb = persist.tile([P, NTB, P], DT)
            rk_tiles = persist.tile([P, NTB], F32)
            wo_sb = persist.tile([P, G, D], DT)
            otn_sb = [persist.tile([P, T], DT, tag=f"otn{h}", name=f"otn{h}")
                      for h in range(G)]

            # PE warm-up: keep PE busy through the cold-clock ramp window
            # while the first x chunks stream in.
            with tc.tile_pool(name="warm", bufs=1, space="PSUM") as ps_w:
                warm_ps = ps_w.tile([1, P], F32)
                for _ in range(20):
                    nc.tensor.matmul(warm_ps, ones_col, warm_src,
                                     start=True, stop=True)

            # ---------------- Phase 1: projections + RoPE + norms -----------
            with tc.tile_pool(name="weights", bufs=1) as wpool, \
                 tc.tile_pool(name="xts", bufs=4) as xpool, \
                 tc.tile_pool(name="p1tmp", bufs=4) as tmpool, \
                 tc.tile_pool(name="p1q", bufs=8) as qpool, \
                 tc.tile_pool(name="p1ps", bufs=3, space="PSUM") as ps_a, \
                 tc.tile_pool(name="p1psv", bufs=2, space="PSUM") as ps_v, \
                 tc.tile_pool(name="p1sc", bufs=1, space="PSUM") as ps_sc, \
                 tc.tile_pool(name="p1sq", bufs=2, space="PSUM") as ps_sq:
                wq_sb = wpool.tile([P, G, NKT, HD], DT)
                wk_sb = wpool.tile([P, NKT, HD], DT)
                wv_sb = wpool.tile([P, NKT, HD], DT)
                nc.sync.dma_start(out=wk_sb, in_=wk)

                xts_tiles = {}

                def get_xts(i):
                    if i in xts_tiles or i >= NTC1:
                        return
                    xts = xpool.tile([P, NKT, TC1], DT, tag="xts",
                                     name="xts")
                    sl_i = slice(i * TC1, (i + 1) * TC1)
                    xre = xT[:, sl_i].rearrange("(kt p) m -> p kt m", p=P)
                    for kg in range(4):
                        nc.sync.dma_start(
                            out=xts[:, 4 * kg:4 * (kg + 1), :],
                            in_=xre[:, 4 * kg:4 * (kg + 1), :])
                    xts_tiles[i] = xts

                # startup staging on the SP DMA queue: PE-critical first,
                # tables behind the second x chunk (only DVE ropes wait)
                get_xts(0)
                for h in range(G):
                    nc.sync.dma_start(out=wq_sb[:, h], in_=wq[:, h])
                nc.sync.dma_start(out=wv_sb, in_=wv)
                get_xts(1)
                nc.sync.dma_start(out=cosk_sb, in_=cosk)
                nc.sync.dma_start(out=sink_sb, in_=sink)
                get_xts(2)
                nc.sync.dma_start(out=cosq_sb, in_=cosq)
                nc.sync.dma_start(out=sinq_sb, in_=sinq)

                def swap_copy(psb, tag):
                    # halves-swapped copy (single-input ops may cross
                    # partition bases; two-input SB+SB ops may not)
                    psb_sw = tmpool.tile([P, TC1], DT, tag=tag, name=tag)
                    nc.vector.tensor_copy(out=psb_sw[0:64, :],
                                          in_=psb[64:128, :])
                    nc.vector.tensor_copy(out=psb_sw[64:128, :],
                                          in_=psb[0:64, :])
                    return psb_sw

                def rope(psb, psb_sw, cos_t, sin_t, dst):
                    # dst = psb * cos + swap(psb) * sin   (all fp16 SBUF,
                    # partition-aligned; sin table carries the sign fold)
                    tmp = tmpool.tile([P, TC1], DT, tag="ropetmp",
                                      name="ropetmp")
                    nc.vector.tensor_mul(out=tmp, in0=psb_sw, in1=sin_t)
                    tmp2 = tmpool.tile([P, TC1], DT, tag="ropetmp2",
                                       name="ropetmp2")
                    nc.vector.tensor_mul(out=tmp2, in0=psb, in1=cos_t)
                    nc.vector.tensor_add(out=dst, in0=tmp2, in1=tmp)

                for tc_i in range(NTC1):
                    sl = slice(tc_i * TC1, (tc_i + 1) * TC1)
                    get_xts(tc_i + 1)
                    if tc_i == 3:
                        nc.sync.dma_start(
                            out=wo_sb,
                            in_=wo.rearrange("(h p) m -> p h m", p=P))
                    xts = xts_tiles.pop(tc_i)

                    # ---- PE: projections (K, Q heads, V) -------------------
                    ps_k = ps_a.tile([P, TC1], F32, tag="proj", name="ps_k")
                    for kt in range(NKT):
                        nc.tensor.matmul(ps_k, wk_sb[:, kt, :],
                                         xts[:, kt, :],
                                         start=(kt == 0), stop=(kt == NKT - 1))
                    # Act: psum -> fp16 SBUF copy + square (pre-RoPE norm)
                    psb_k = tmpool.tile([P, TC1], DT, tag="psb", name="psb_k")
                    nc.scalar.copy(out=psb_k, in_=ps_k)
                    psw_k = swap_copy(psb_k, "psw")
                    sq_k = tmpool.tile([P, TC1], DT, tag="sq", name="sq_k")
                    nc.scalar.square(out=sq_k, in_=psb_k)
                    rope(psb_k, psw_k, cosk_sb[:, sl], sink_sb[:, sl],
                         kT_sb[:, sl])

                    q_ps = []
                    for h in range(G):
                        ps_q = ps_a.tile([P, TC1], F32, tag="proj",
                                         name="ps_q")
                        for kt in range(NKT):
                            nc.tensor.matmul(
                                ps_q, wq_sb[:, h, kt, :],
                                xts[:, kt, :],
                                start=(kt == 0), stop=(kt == NKT - 1))
                        psb_q = qpool.tile([P, TC1], DT, tag="psbq",
                                           name="psb_q")
                        nc.scalar.copy(out=psb_q, in_=ps_q)
                        psw_q = swap_copy(psb_q, "pswq")
                        sq_q = qpool.tile([P, TC1], DT, tag="sqq",
                                          name="sq_q")
                        nc.scalar.square(out=sq_q, in_=psb_q)
                        qr = qpool.tile([P, TC1], DT, tag="ropeq", name="qr")
                        rope(psb_q, psw_q, cosq_sb[:, sl], sinq_sb[:, sl], qr)
                        q_ps.append((sq_q, qr))

                    # V directly in [t, d] layout: x block stationary
                    for i in range(TC1 // P):
                        ps_vt = ps_v.tile([P, P], F32, tag="vt", name="ps_vt")
                        for kt in range(NKT):
                            nc.tensor.matmul(
                                ps_vt, xts[:, kt, i * P:(i + 1) * P],
                                wv_sb[:, kt, :],
                                start=(kt == 0), stop=(kt == NKT - 1))
                        nc.scalar.copy(
                            out=v_sb[:, tc_i * (TC1 // P) + i, :], in_=ps_vt)

                    # ---- norm reductions (PE, end of chunk) ----------------
                    # K: per key-block column sums of sq_k (sq stationary)
                    kb0 = tc_i * (TC1 // P)
                    ssqc = ps_sc.tile([P, TC1 // P], F32, tag="ssqc",
                                      name="ssqc")
                    for i in range(TC1 // P):
                        nc.tensor.matmul(ssqc[:, i:i + 1],
                                         sq_k[:, i * P:(i + 1) * P],
                                         ones_col, start=True, stop=True)
                    nc.scalar.activation(
                        out=rk_tiles[:, kb0:kb0 + TC1 // P], in_=ssqc,
                        func=AF.Sqrt, bias=eps_k[:], scale=float(1.0 / HD))
                    nc.vector.reciprocal(
                        out=rk_tiles[:, kb0:kb0 + TC1 // P],
                        in_=rk_tiles[:, kb0:kb0 + TC1 // P])
                    # Q: rq = 1/sqrt(ssq + HD*eps) applied to q columns
                    for h in range(G):
                        sq_q, qr = q_ps[h]
                        ssq = ps_sq.tile([1, TC1], F32, tag="ssq",
                                         name="ssq")
                        nc.tensor.matmul(ssq, ones_col, sq_q,
                                         start=True, stop=True)
                        sq_s = stri.tile([1, TC1], F32, tag="sqs",
                                         name="sq_s")
                        nc.scalar.activation(out=sq_s, in_=ssq, func=AF.Sqrt,
                                             bias=eps_q[:], scale=1.0)
                        rq = stri.tile([1, TC1], DT, tag="rq", name="rq")
                        nc.vector.reciprocal(out=rq, in_=sq_s)
                        rqB = tmpool.tile([P, TC1], DT, tag="rqB", name="rqB")
                        nc.gpsimd.partition_broadcast(rqB, rq)
                        nc.vector.tensor_mul(out=q_sb[h][:, sl], in0=qr,
                                             in1=rqB)

            # ---------------- Phase 2: attention + c_proj -------------------
            with tc.tile_pool(name="attn", bufs=4) as apool, \
                 tc.tile_pool(name="pb", bufs=6) as ppool, \
                 tc.tile_pool(name="pacc", bufs=4) as accpool, \
                 tc.tile_pool(name="ysb", bufs=4) as ypool, \
                 tc.tile_pool(name="p2st", bufs=2, space="PSUM") as ps_st, \
                 tc.tile_pool(name="p2ot", bufs=2, space="PSUM") as ps_ot, \
                 tc.tile_pool(name="p3ya", bufs=1, space="PSUM") as ps_ya, \
                 tc.tile_pool(name="p3yb", bufs=1, space="PSUM") as ps_yb:

                def cproj_steps(qc):
                    # 8 emission closures for q-chunk qc's 4 t-blocks
                    steps = []
                    for tb in range(4 * qc, 4 * qc + 4):
                        for jg in (0, 2):
                            def step(tb=tb, jg=jg):
                                ya = ps_ya.tile([P, TC], F32, tag="ya",
                                                name="ya")
                                yb = ps_yb.tile([P, TC], F32, tag="yb",
                                                name="yb")
                                for h in range(G):
                                    lhs = otn_sb[h][:, tb * P:(tb + 1) * P]
                                    nc.tensor.matmul(
                                        ya, lhs,
                                        wo_sb[:, h, jg * TC:(jg + 1) * TC],
                                        start=(h == 0), stop=(h == G - 1))
                                    nc.tensor.matmul(
                                        yb, lhs,
                                        wo_sb[:, h,
                                              (jg + 1) * TC:(jg + 2) * TC],
                                        start=(h == 0), stop=(h == G - 1))
                                for j, yp in ((jg, ya), (jg + 1, yb)):
                                    y_sb = ypool.tile([P, TC], DT, tag="y_sb",
                                                      name="y_sb")
                                    nc.vector.tensor_copy(out=y_sb, in_=yp)
                                    nc.sync.dma_start(
                                        out=y[tb * P:(tb + 1) * P,
                                              j * TC:(j + 1) * TC],
                                        in_=y_sb)
                            steps.append(step)
                    return steps

                for qc in range(NTC):
                    qsl = slice(qc * TC, (qc + 1) * TC)
                    nkb = 4 * (qc + 1)
                    steps = cproj_steps(qc - 1) if qc > 0 else []
                    # interleave points: 2 mid-loop per pair + pair ends
                    mids = {max(1, nkb // 3), max(2, (2 * nkb) // 3)}
                    for pair in ((0, 1), (2, 3)):
                        ot_ps = {}
                        for h in pair:
                            ot_ps[h] = ps_ot.tile([P, TC], F32, tag="ot",
                                                  name="ot_ps")
                        acc = accpool.tile([P, 2, TC], DT, tag="acc",
                                           name="acc")
                        for kb in range(nkb):
                            r = kb - 4 * qc  # >=0 on diagonal blocks
                            c0 = max(r, 0) * P  # first valid q column
                            # both heads' scores into one psum pair-tile
                            st = ps_st.tile([P, 2, TC], F32, tag="st",
                                            name="st_ps")
                            for i, h in enumerate(pair):
                                nc.tensor.matmul(
                                    st[:, i, c0:],
                                    kT_sb[:, kb * P:(kb + 1) * P],
                                    q_sb[h][:, qc * TC + c0:(qc + 1) * TC],
                                    start=True, stop=True)
                            # one exp over both heads (shared rk scale)
                            p_ = ppool.tile([P, 2, TC], DT, tag="p",
                                            name="p_sb")
                            nc.scalar.activation(
                                out=p_[:, :, c0:], in_=st[:, :, c0:],
                                func=AF.Exp, bias=expb[:],
                                scale=rk_tiles[:, kb:kb + 1])
                            if r >= 0:
                                for i in range(2):
                                    # causal mask on the diagonal strip
                                    nc.gpsimd.affine_select(
                                        out=p_[:, i, c0:c0 + P],
                                        in_=p_[:, i, c0:c0 + P],
                                        pattern=[[1, P]],
                                        compare_op=ALU.is_ge,
                                        fill=0.0,
                                        base=0,
                                        channel_multiplier=-1)
                            if kb == 0:
                                nc.vector.tensor_copy(out=acc, in_=p_)
                            else:
                                nc.vector.tensor_add(
                                    out=acc[:, :, c0:],
                                    in0=acc[:, :, c0:],
                                    in1=p_[:, :, c0:])
                            for i, h in enumerate(pair):
                                nc.tensor.matmul(
                                    ot_ps[h][:, c0:], v_sb[:, kb, :],
                                    p_[:, i, c0:], start=(kb == 0),
                                    stop=(kb == nkb - 1))
                            if steps and (kb in mids):
                                steps.pop(0)()
                        # pair rowsum -> 1/rowsum -> normalize (rs tile
                        # borrows a st-pool buffer; scores are done)
                        rs = ps_st.tile([P, 2, TC], F32, tag="st",
                                        name="rs_ps")
                        for i in range(2):
                            nc.tensor.matmul(rs[0:1, i, :], ones_col,
                                             acc[:, i, :],
                                             start=True, stop=True)
                        recip = stri.tile([1, 2, TC], DT, tag="recip",
                                          name="recip")
                        nc.vector.reciprocal(out=recip, in_=rs[0:1, :, :])
                        recipB = apool.tile([P, 2, TC], DT, tag="recipB",
                                            name="recipB")
                        nc.gpsimd.partition_broadcast(recipB, recip)
                        if steps:
                            steps.pop(0)()
                        for i, h in enumerate(pair):
                            nc.vector.tensor_mul(out=otn_sb[h][:, qsl],
                                                 in0=ot_ps[h],
                                                 in1=recipB[:, i, :])
                        if steps:
                            steps.pop(0)()
                    while steps:
                        steps.pop(0)()
                # final chunk's c_proj
                for step in cproj_steps(NTC - 1):
                    step()

    nc.compile()
    return nc


_NC_CACHE = None


def _get_program():
    global _NC_CACHE
    if _NC_CACHE is None:
        _NC_CACHE = build_program()
    return _NC_CACHE


def _make_tables(pos, gamma2):
    half = HD // 2
    inv_freq = 1.0 / (THETA ** (np.arange(half, dtype=np.float64) / half))
    ang = (pos + np.arange(T, dtype=np.float64))[None, :] * inv_freq[:, None]
    cos = np.cos(ang)
    sin = np.sin(ang)
    cosq = np.concatenate([cos, cos], axis=0)
    sinq = np.concatenate([-sin, sin], axis=0)
    g2 = gamma2.astype(np.float64).reshape(P, 1)
    return (np.ascontiguousarray(cosq.astype(NP_DT)),
            np.ascontiguousarray(sinq.astype(NP_DT)),
            np.ascontiguousarray((cosq * g2).astype(NP_DT)),
            np.ascontiguousarray((sinq * g2).astype(NP_DT)))


def kernel(x, Wq, Wk, Wv, Wo, q_gamma, k_gamma, pos):
    x = np.asarray(x, dtype=np.float32)
    Wq = np.asarray(Wq, dtype=np.float32)
    Wk = np.asarray(Wk, dtype=np.float32)
    Wv = np.asarray(Wv, dtype=np.float32)
    Wo = np.asarray(Wo, dtype=np.float32)
    q_gamma = np.asarray(q_gamma, dtype=np.float32)
    k_gamma = np.asarray(k_gamma, dtype=np.float32)
    pos = int(np.asarray(pos))

    gamma2 = q_gamma * k_gamma
    cosq, sinq, cosk, sink = _make_tables(pos, gamma2)

    def st(a):
        return np.ascontiguousarray(a.astype(NP_DT))

    def wqr(a):
        # (D, G*HD) -> (P, G, NKT, HD): [p, h, kt, :] = a[kt*P + p, h*HD:]
        return st(a.reshape(NKT, P, G, HD).transpose(1, 2, 0, 3))

    def wkvr(a):
        # (D, HD) -> (P, NKT, HD)
        return st(a.reshape(NKT, P, HD).transpose(1, 0, 2))

    xTs = [st(x[b].T) for b in range(B)]
    in_maps = []
    for c in range(N_CORES):
        b, n = divmod(c, NKV)
        in_maps.append({
            "xT": xTs[b],
            "wq": wqr(Wq[:, n * G * HD:(n + 1) * G * HD]),
            "wk": wkvr(Wk[:, n * HD:(n + 1) * HD]),
            "wv": wkvr(Wv[:, n * HD:(n + 1) * HD]),
            "wo": st(Wo[n * G * HD:(n + 1) * G * HD, :]),
            "cosq": cosq,
            "sinq": sinq,
            "cosk": cosk,
            "sink": sink,
        })

    nc = _get_program()
    res = bass_utils.run_bass_kernel_spmd(nc, in_maps,
                                          core_ids=list(range(N_CORES)))
    out = np.zeros((B, T, D), dtype=np.float32)
    for c in range(N_CORES):
        b = c // NKV
        out[b] += res.results[c]["y"].astype(np.float32)
    return out


if __name__ == "__main__":
    build_program()
    print("program built OK")
